# revision 1
# baseline (speedup 1.0000x reference)
"""Bidirectional GRU (AbstractBiRNN) Trainium2 Bass kernel.

Problem: B=32, T=512, D=U=512, fp32 in/out.
    outs_f = GRU_scan(x, Wf, Uf, bf)          # forward over t
    outs_b = GRU_scan(x[:, ::-1], Wb, Ub, bb) # backward (scan order kept)
    out = concat([outs_f, outs_b], axis=-1)   # [B, T, 2U]

Strategy (8 NeuronCores, zero inter-core communication):
  - core c: direction d = c//4 (0=fwd, 1=bwd), batch shard s = c%4 (rows 8s..8s+8).
  - Each core projects its own x shard (x @ W', fp32r matmuls at N=512, full rate)
    directly into SBUF in a gate-transposed layout, then runs the 512-step GRU
    scan locally.
  - Scan layout is "U-major folded": every per-step tensor lives in one
    [128 partitions, n_slices, 8 batch] SBUF tile (partition = U-dim within a
    128-slice). Matmuls keep recurrent weights stationary (lhsT = U_rec block
    [128,128]) and stream h as the N=8 moving operand, so gate outputs come out
    already U-major and no per-step transpose is ever needed.
  - hard_sigmoid(v) = clip(0.2 v + 0.5, 0, 1) is folded into the weights:
    W'_zr = 0.2 Wzr, b'_zr = 0.2 bzr + 0.5, U'_zr = 0.2 Uzr, so per step
    z,r = clip(psum + xg', 0, 1) -- two fused DVE ops.
  - Projection of chunk ts+1 is interleaved between scan steps of chunk ts to
    fill PE idle slots.

Host folds/reassembles layouts; all hot-loop compute is on-device.
"""

import os
import numpy as np
import ml_dtypes

import concourse.bass as bass
import concourse.tile as tile
from concourse import bacc, mybir
from concourse.bass_utils import run_bass_kernel_spmd
from concourse.tile_rust import add_dep_helper

F32 = mybir.dt.float32
F32R = mybir.dt.float32r
BF16 = mybir.dt.bfloat16
F16 = mybir.dt.float16
AF = mybir.ActivationFunctionType
OP = mybir.AluOpType

B, T, D, U = 32, 512, 512, 512
NCORE = 8
BP = B // 4          # batch rows per core (4 shards per direction) = 8
CHUNK = 64           # scan steps per xg chunk resident in SBUF
KD = D // 128        # k-slices of contraction (4)
GZ = (2 * U) // 128  # zr gate slices (8)
GH = U // 128        # cand gate slices (4)
G = GZ + GH          # total gate slices (12)

# Scan matmul dtype: bf16 => fast weight loads (FWL); fp32 => exact, ~2x slower
SCAN_BF16 = os.environ.get("GRU_SCAN_DT", "bf16") == "bf16"
SDT = BF16 if SCAN_BF16 else F32
SDT_NP = ml_dtypes.bfloat16 if SCAN_BF16 else np.float32
# xg via PSUM identity-matmul preloads (1) vs DVE adds (0)
PRELOAD = os.environ.get("GRU_PRELOAD", "1") == "1"


def _build(t_steps=T, reps=1, ablate="none"):
    """Emit the SPMD program (identical for all cores; data differs).

    ablate: "none" | "mm_only" (scan matmuls only, no elementwise chain)
          | "chain_only" (elementwise chain, each MM group shrunk to 1 MM).
    """
    nch = t_steps // CHUNK
    ntok = BP * t_steps

    nc = bacc.Bacc("TRN2", target_bir_lowering=False, debug=False,
                   num_devices=NCORE)

    # Robustness preamble: with target_bir_lowering=False Bass skips its
    # stale-semaphore reset, so a previously killed execution would poison
    # every later run on the same cores. Emit the same reset by hand.
    for sem_range in bass.compact_to_ranges(
            [s for s in nc._kernel_sem_range if s not in nc.barrier_sems]):
        nc.gpsimd.dma_reset(sem_range)
        nc.gpsimd.sem_clear(sem_range)
    nc._nrt_pseudo_barrier()

    # DRAM I/O (per core). xT[p, k, tau] = x[b, t, 128k+p], tau = t*BP + b.
    xT_d = nc.dram_tensor("xT", [128, KD, ntok], F32R, kind="ExternalInput").ap()
    wp_d = nc.dram_tensor("Wp", [128, KD, G * 128], F32R, kind="ExternalInput").ap()
    bias_d = nc.dram_tensor("bias", [128, G], F32, kind="ExternalInput").ap()
    uzr_d = nc.dram_tensor("Uzr", [128, KD, GZ * 128], SDT, kind="ExternalInput").ap()
    uh_d = nc.dram_tensor("Uh", [128, KD, GH * 128], SDT, kind="ExternalInput").ap()
    ident_d = nc.dram_tensor("ident", [128, 128], F16, kind="ExternalInput").ap()
    # outT[ts, p, t_in, s, b] = h_{ts*CHUNK+t_in}[b, 128s+p]
    out_d = nc.dram_tensor("outT", [nch, 128, CHUNK, GH, BP], F32,
                           kind="ExternalOutput").ap()
    dbg_d = None
    if os.environ.get("GRU_DEBUG") == "1":
        dbg_d = nc.dram_tensor("dbg", [128, G, CHUNK * BP], F16,
                               kind="ExternalOutput").ap()
    dump_d = None
    if ablate == "mm_only":
        dump_d = nc.dram_tensor("dump", [2, 128, GZ, BP], F32,
                                kind="ExternalOutput").ap()

    with tile.TileContext(nc) as tc:
        with (
            tc.tile_pool(name="singles", bufs=1) as singles,
            tc.tile_pool(name="xtc", bufs=2) as xtcp,
            tc.tile_pool(name="chunks", bufs=2) as chunks,
            tc.tile_pool(name="outs", bufs=2) as outs,
            tc.tile_pool(name="step", bufs=3) as stepp,
            tc.tile_pool(name="ps_zr", bufs=2, space="PSUM") as ps_zr,
            tc.tile_pool(name="ps_r", bufs=2, space="PSUM") as ps_r,
            tc.tile_pool(name="ps_c", bufs=2, space="PSUM") as ps_c,
            tc.tile_pool(name="ps_p", bufs=2, space="PSUM") as ps_p,
        ):
            # ---- resident tensors ----
            wp = singles.tile([128, KD, G * 128], F32R)
            bias = singles.tile([128, G], F32)
            uzr = singles.tile([128, KD, GZ * 128], SDT)
            uh = singles.tile([128, KD, GH * 128], SDT)
            nc.sync.dma_start(out=wp, in_=wp_d)
            nc.sync.dma_start(out=bias, in_=bias_d)
            nc.sync.dma_start(out=uzr, in_=uzr_d)
            nc.sync.dma_start(out=uh, in_=uh_d)

            ident = singles.tile([128, 128], F16)
            nc.sync.dma_start(out=ident, in_=ident_d)

            proj_state = {}
            acc1 = acc2 = None
            if ablate == "mm_only":
                acc1 = singles.tile([128, GZ, BP], F32)
                acc2 = singles.tile([128, GH, BP], F32)
                nc.vector.memset(acc1, 0.0)
                nc.vector.memset(acc2, 0.0)

            h0_f = singles.tile([128, GH, BP], F32)
            nc.vector.memset(h0_f, 0.0)
            h0_m = h0_f
            if SCAN_BF16:
                h0_m = singles.tile([128, GH, BP], BF16)
                nc.vector.memset(h0_m, 0.0)

            def fetch_xtc(ts):
                """Stream this chunk's x.T slice into SBUF."""
                xtc = xtcp.tile([128, KD, CHUNK * BP], F32R)
                nc.sync.dma_start(
                    out=xtc,
                    in_=xT_d[:, :, CHUNK * BP * ts:CHUNK * BP * (ts + 1)])
                return xtc

            def emit_proj(g, xtc, ct, k=None):
                """xg'[:, g, chunk] = x @ Wp[:, g-slice] + bias, into SBUF.

                With k given, emits only that single K-pass matmul (the psum
                tile is threaded via proj_state); bias-add lands on DVE after
                the last pass (keeps ACT mono-function: Tanh only).
                """
                if k is None:
                    ks = range(KD)
                else:
                    ks = [k]
                if k is None or k == 0:
                    pp = ps_p.tile([128, CHUNK * BP], F32, tag="projps")
                    proj_state[g] = pp
                pp = proj_state[g]
                for kk in ks:
                    nc.tensor.matmul(
                        out=pp[:],
                        lhsT=wp[:, kk, 128 * g:128 * (g + 1)],
                        rhs=xtc[:, kk, :],
                        start=(kk == 0), stop=(kk == KD - 1))
                if k is None or k == KD - 1:
                    # split into 4 slices so a bias-add can never occupy DVE
                    # for a long 512-wide block on the critical chain
                    tc.cur_priority += 50000
                    q = CHUNK * BP // 4
                    for i in range(4):
                        nc.vector.tensor_scalar_add(
                            ct[:, g, q * i:q * (i + 1)],
                            pp[:, q * i:q * (i + 1)], bias[:, g:g + 1])
                    tc.cur_priority -= 50000

            def scan_step(ct, ot, t_in, hp_f, hp_m):
                """One GRU step. hp_f/hp_m: previous h (fp32 AP / matmul-dtype AP).
                Returns (h_f32_ap, h_mm_ap) for the next step.

                xz/xr/xh are preloaded into PSUM via identity matmuls
                (start=True sets has_written properly), the gate matmuls
                accumulate on top, and clip/tanh read PSUM directly.
                r lives in its own bank and its m-groups run first so the
                clip-r -> rh -> MM2 chain starts while z m-groups stream.
                """
                xg = ct[:, :, BP * t_in:BP * (t_in + 1)]
                z_ps = ps_zr.tile([128, GH, BP], F32)
                r_ps = ps_r.tile([128, GH, BP], F32)
                c_ps = ps_c.tile([128, GH, BP], F32)
                # xg preloads: fp16 identity matmuls (start=True sets
                # has_written; banks are tile-exclusive so nothing clears them)
                pre = not (ablate == "chain_only")
                pre_r = pre_c = pre_z = None
                if PRELOAD:
                    pre_r = nc.tensor.matmul(out=r_ps[:], lhsT=ident,
                                             rhs=xg[:, GH:GZ, :], start=True,
                                             stop=not pre)
                    pre_c = nc.tensor.matmul(out=c_ps[:], lhsT=ident,
                                             rhs=xg[:, GZ:G, :], start=True,
                                             stop=not pre)
                    pre_z = nc.tensor.matmul(out=z_ps[:], lhsT=ident,
                                             rhs=xg[:, 0:GH, :], start=True,
                                             stop=not pre)

                def _ordered_mm(pre_inst, out, lhsT, rhs, k, is_first):
                    """Accumulating matmul; the bank-clearing preload MUST
                    precede it in PE order -- Tile treats same-psum matmuls
                    as commutative, so pin the order explicitly."""
                    mm = nc.tensor.matmul(out=out, lhsT=lhsT, rhs=rhs,
                                          start=(k == 0 and not PRELOAD),
                                          stop=(k == KD - 1))
                    if PRELOAD:
                        add_dep_helper(mm.ins, pre_inst.ins, sync=False,
                                       reason="accumulate after xg preload")
                    return mm
                # r m-groups first: the critical path (r -> rh -> MM2) starts
                # while the z m-groups are still streaming on PE.
                mm1r = [(m, k) for m in range(GH, GZ) for k in range(KD)]
                mm1z = [(m, k) for m in range(GH) for k in range(KD)]
                if ablate == "chain_only":
                    mm1r, mm1z = [], []
                for m, k in mm1r:
                    _ordered_mm(pre_r, r_ps[:, m - GH, :],
                                uzr[:, k, 128 * m:128 * (m + 1)],
                                hp_m[:, k, :], k, k == 0)
                if ablate == "mm_only":
                    for m, k in mm1z:
                        _ordered_mm(pre_z, z_ps[:, m, :],
                                    uzr[:, k, 128 * m:128 * (m + 1)],
                                    hp_m[:, k, :], k, k == 0)
                    nc.vector.tensor_tensor(acc1[:, 0:GH, :], acc1[:, 0:GH, :],
                                            z_ps, op=OP.max)
                    nc.vector.tensor_tensor(acc1[:, GH:GZ, :], acc1[:, GH:GZ, :],
                                            r_ps, op=OP.max)
                    for m in range(GH):
                        for k in range(KD):
                            _ordered_mm(pre_c, c_ps[:, m, :],
                                        uh[:, k, 128 * m:128 * (m + 1)],
                                        h0_m[:, k, :], k, k == 0)
                    nc.vector.tensor_tensor(acc2, acc2, c_ps, op=OP.max)
                    return hp_f, hp_m
                # critical: r = clip(psum [+ xr], 0, 1), rh = r*h
                r_sb = stepp.tile([128, GH, BP], F32)
                if not PRELOAD:
                    nc.vector.scalar_tensor_tensor(
                        out=r_sb, in0=r_ps, scalar=0.0, in1=xg[:, GH:GZ, :],
                        op0=OP.bypass, op1=OP.add)
                    nc.vector.tensor_scalar(
                        out=r_sb, in0=r_sb, scalar1=1.0, scalar2=0.0,
                        op0=OP.min, op1=OP.max)
                else:
                    nc.vector.tensor_scalar(
                        out=r_sb, in0=r_ps, scalar1=1.0, scalar2=0.0,
                        op0=OP.min, op1=OP.max)
                rh = stepp.tile([128, GH, BP], SDT)
                rh_inst = nc.vector.tensor_tensor(rh, r_sb, hp_f, op=OP.mult)
                # z m-groups after the r-chain kickoff
                for m, k in mm1z:
                    _ordered_mm(pre_z, z_ps[:, m, :],
                                uzr[:, k, 128 * m:128 * (m + 1)],
                                hp_m[:, k, :], k, k == 0)
                mm2 = [(m, k) for m in range(GH) for k in range(KD)]
                if ablate == "chain_only":
                    mm2 = []
                for m, k in mm2:
                    _ordered_mm(pre_c, c_ps[:, m, :],
                                uh[:, k, 128 * m:128 * (m + 1)],
                                rh[:, k, :], k, k == 0)
                # off the critical path, overlap MM2: z, z*h, 1-z
                # (priority-demoted so the scheduler never favors these over
                # the critical clip-r -> rh -> MM2 -> tanh chain)
                tc.cur_priority += 50000
                z_sb = stepp.tile([128, GH, BP], F32)
                if not PRELOAD:
                    nc.vector.scalar_tensor_tensor(
                        out=z_sb, in0=z_ps, scalar=0.0, in1=xg[:, 0:GH, :],
                        op0=OP.bypass, op1=OP.add)
                    clipz_inst = nc.vector.tensor_scalar(
                        out=z_sb, in0=z_sb, scalar1=1.0, scalar2=0.0,
                        op0=OP.min, op1=OP.max)
                else:
                    clipz_inst = nc.vector.tensor_scalar(
                        out=z_sb, in0=z_ps, scalar1=1.0, scalar2=0.0,
                        op0=OP.min, op1=OP.max)
                # greedy scheduler guard: never let clip-z occupy DVE between
                # clip-r and rh (order-only edge, no semaphore)
                add_dep_helper(clipz_inst.ins, rh_inst.ins, sync=False,
                               reason="z-side after critical rh")
                zh = stepp.tile([128, GH, BP], F32)
                nc.vector.tensor_tensor(zh, z_sb, hp_f, op=OP.mult)
                omz = stepp.tile([128, GH, BP], F32)
                nc.vector.tensor_scalar(out=omz, in0=z_sb,
                                        scalar1=-1.0, scalar2=1.0,
                                        op0=OP.mult, op1=OP.add)
                tc.cur_priority -= 50000
                # critical tail: cand = tanh(c_ps [+ xh]); h = zh + (1-z)*cand
                cand = stepp.tile([128, GH, BP], F32)
                if not PRELOAD:
                    tmp = stepp.tile([128, GH, BP], F32)
                    nc.vector.scalar_tensor_tensor(
                        out=tmp, in0=c_ps, scalar=0.0, in1=xg[:, GZ:G, :],
                        op0=OP.bypass, op1=OP.add)
                    nc.scalar.activation(cand, tmp, AF.Tanh)
                else:
                    nc.scalar.activation(cand, c_ps, AF.Tanh)
                t2 = stepp.tile([128, GH, BP], F32)
                nc.vector.tensor_tensor(t2, omz, cand, op=OP.mult)
                hn = ot[:, t_in, :, :]
                if SCAN_BF16:
                    # bf16 h for next matmul computed as an independent second
                    # add (parallel to the fp32 one) -- no dependent cast hop
                    hb = stepp.tile([128, GH, BP], BF16)
                    nc.vector.tensor_tensor(hb, zh, t2, op=OP.add)
                    nc.vector.tensor_tensor(hn, zh, t2, op=OP.add)
                    return hn, hb
                nc.vector.tensor_tensor(hn, zh, t2, op=OP.add)
                return hn, hn

            hp_f, hp_m = h0_f, h0_m
            for rep in range(reps):
                xtc_cur = fetch_xtc(0)
                ct_cur = chunks.tile([128, G, CHUNK * BP], F16)
                for g in range(G):
                    emit_proj(g, xtc_cur, ct_cur)
                if dbg_d is not None and rep == 0:
                    nc.sync.dma_start(out=dbg_d, in_=ct_cur)
                # reps>1 (timing builds only): chain h across reps so DCE
                # cannot eliminate earlier reps
                for ts in range(nch):
                    ot = outs.tile([128, CHUNK, GH, BP], F32)
                    pq = []
                    ct_next = xtc_next = None
                    if ts + 1 < nch:
                        xtc_next = fetch_xtc(ts + 1)
                        ct_next = chunks.tile([128, G, CHUNK * BP], F16)
                        pq = [(g, k) for g in range(G) for k in range(KD)]
                    for t_in in range(CHUNK):
                        hp_f, hp_m = scan_step(ct_cur, ot, t_in, hp_f, hp_m)
                        if pq:
                            pg, pk = pq.pop(0)
                            emit_proj(pg, xtc_next, ct_next, k=pk)
                    if ablate != "mm_only":
                        nc.sync.dma_start(out=out_d[ts], in_=ot)
                    ct_cur, xtc_cur = ct_next, xtc_next
            if ablate == "mm_only":
                nc.sync.dma_start(out=dump_d[0], in_=acc1)
                nc.sync.dma_start(out=dump_d[1][:, :GH, :], in_=acc2)

    nc.compile()
    return nc


_CACHE = {}


def _get_nc(t_steps=T, reps=1, ablate="none"):
    key = (t_steps, reps, SCAN_BF16, ablate, PRELOAD)
    if key not in _CACHE:
        _CACHE[key] = _build(t_steps, reps, ablate)
    return _CACHE[key]


def _prep_inputs(x, Wf, Uf, bf, Wb, Ub, bb, t_steps=T):
    """Build per-core in_maps (host-side fold of scales + layouts)."""
    x = np.asarray(x, dtype=np.float32)
    in_maps = []
    for c in range(NCORE):
        d, s = divmod(c, 4)
        W = np.asarray(Wf if d == 0 else Wb, np.float32)
        Urec = np.asarray(Uf if d == 0 else Ub, np.float32)
        bvec = np.asarray(bf if d == 0 else bb, np.float32)
        rows = slice(BP * s, BP * (s + 1))
        xr = x[rows, :t_steps, :]                     # [BP, t, D]
        if d == 1:
            xr = xr[:, ::-1, :]
        # xT[p, k, tau] = xr[b, t, 128k+p]
        xT = np.ascontiguousarray(
            xr.transpose(2, 1, 0).reshape(KD, 128, t_steps * BP)
              .transpose(1, 0, 2))
        # folded projection weights / bias (hard_sigmoid affine into zr part)
        Wp = W.copy()
        Wp[:, :2 * U] *= 0.2
        bp = bvec.copy()
        bp = np.concatenate([0.2 * bp[:2 * U] + 0.5, bp[2 * U:]])
        WpT = np.ascontiguousarray(
            Wp.reshape(KD, 128, G * 128).transpose(1, 0, 2))
        biasT = np.ascontiguousarray(
            bp.reshape(G, 128).transpose(1, 0))
        Uzr = np.ascontiguousarray(
            (0.2 * Urec[:, :2 * U]).reshape(KD, 128, GZ * 128)
            .transpose(1, 0, 2)).astype(SDT_NP)
        Uh = np.ascontiguousarray(
            Urec[:, 2 * U:].reshape(KD, 128, GH * 128)
            .transpose(1, 0, 2)).astype(SDT_NP)
        in_maps.append({
            "ident": np.eye(128, dtype=np.float16),
            "xT": xT.astype(np.float32),
            "Wp": WpT.astype(np.float32),
            "bias": biasT.astype(np.float32),
            "Uzr": Uzr,
            "Uh": Uh,
        })
    return in_maps


def _assemble(results, t_steps=T):
    out = np.empty((B, t_steps, 2 * U), np.float32)
    for c in range(NCORE):
        d, s = divmod(c, 4)
        arr = results[c]["outT"]                      # [nch,128,CHUNK,GH,BP]
        blk = arr.transpose(4, 0, 2, 3, 1).reshape(BP, t_steps, U)
        out[BP * s:BP * (s + 1), :, d * U:(d + 1) * U] = blk
    return out


def kernel(x, Wf, Uf, bf, Wb, Ub, bb):
    nc = _get_nc()
    in_maps = _prep_inputs(x, Wf, Uf, bf, Wb, Ub, bb)
    res = run_bass_kernel_spmd(nc, in_maps, core_ids=list(range(NCORE)))
    return _assemble(res.results)

